# revision 15
# baseline (speedup 1.0000x reference)
"""NetVLAD layer kernel for Trainium2 (Bass/Tile), data-parallel over batch on 8 cores.

Math (per image):
  s = x @ Wk + bias          # [HW, K]   x:[HW, D], Wk:[D, K]
  a = softmax(s, axis=-1)    # [HW, K]
  vT[k, d] = sum_p a[p,k] x[p,d] + (sum_p a[p,k]) * C[d,k]
  intra L2-normalize over d -> global L2-normalize -> out [K*D]

Sharding: batch 32 -> 4 images per core; Wk/bias/C replicated.

Layouts/precision (v2 design):
  - Host ships TWO layouts of x: xt (D-major, bf16, pre-tiled for the
    s-matmul stationary) and xf (pixel-major, fp8 e4m3, for the DoubleRow
    v-matmul). No on-chip transposes at all.
  - s-path all bf16 (softmax logits are precision-critical; fp8 fails).
  - v-path fp8: a is quantized to fp8 scaled by 64 (dodges e4m3 subnormals;
    the per-row L2 normalization cancels the scale exactly). v-matmul runs
    in DoubleRow mode: one matmul per pixel-chunk PAIR (2x128 contraction).
"""

import sys

sys.path.insert(0, "/opt/trn_rl_repo")

import numpy as np
import ml_dtypes

import concourse.bacc as bacc
import concourse.bass as bass
import concourse.mybir as mybir
import concourse.tile as tile
from concourse import bass_utils

F32 = mybir.dt.float32
BF16 = mybir.dt.bfloat16
FP8 = mybir.dt.float8e4

N_CORES = 8
B = 32
H, W_IMG, D, K = 60, 80, 512, 64
HW = H * W_IMG            # 4800 pixels per image
B_LOC = B // N_CORES      # 4 images per core
P = 128                   # partition / pixel-chunk size
NDC = D // P              # 4 D-chunks
SUP = 4 * P               # 512-pixel super-chunk
NSUP = (HW + SUP - 1) // SUP          # 10 supers (last partial: 192 px)
CHUNKS = [(i * P, min(P, HW - i * P)) for i in range((HW + P - 1) // P)]
NCH = len(CHUNKS)         # 38 chunks: 37 full + one of 64
NPAIR = (NCH + 1) // 2    # 19 chunk-pairs for the DoubleRow v-matmul

A_SCALE = 64.0            # fp8 quantization scale for a (cancels in norm)
EPS = 1e-12


class _patched_act_tables:
    """Context manager: force the act-table-load pass to use the one set that
    contains Exp, Ln and Copy, so the kernel never swaps ACT tables. Restores
    the original lookup on exit (it is global concourse state)."""

    def __enter__(self):
        from concourse import hw_specs
        import functools

        self._orig_hw = hw_specs.get_activation_tables
        self._orig_bacc = bacc.get_activation_tables

        orig = self._orig_hw

        @functools.cache
        def patched(arch):
            tabs = dict(orig(arch))
            if "natural_log_exp_and_others" in tabs:
                tabs = {
                    name: (s if name == "natural_log_exp_and_others" else set())
                    for name, s in tabs.items()
                }
            return tabs

        hw_specs.get_activation_tables = patched
        bacc.get_activation_tables = patched

    def __exit__(self, *exc):
        from concourse import hw_specs

        hw_specs.get_activation_tables = self._orig_hw
        bacc.get_activation_tables = self._orig_bacc
        return False


def build_netvlad(reps: int = 1):
    with _patched_act_tables():
        return _build_netvlad_inner(reps)


def _build_netvlad_inner(reps: int):
    nc = bacc.Bacc("TRN2", target_bir_lowering=False, debug=False, num_devices=N_CORES)

    # xt: D-major bf16, pre-tiled per super-chunk: [b, sup, p_d, c_d*512 + q_pix]
    #     element = x[b, pixel=sup*512+q_pix, d=c_d*128+p_d]
    xt_d = nc.dram_tensor(
        "xt", [B_LOC, NSUP, P, NDC * SUP // 1], BF16, kind="ExternalInput"
    ).ap()
    # xf: pixel-major fp8, pre-tiled: [b, sup, p_pix, c_pix*512 + q_d]
    #     element = x[b, pixel=sup*512+c_pix*128+p_pix, d=q_d] (zero-padded tail)
    xf_d = nc.dram_tensor(
        "xf", [B_LOC, NSUP, P, 4 * D], FP8, kind="ExternalInput"
    ).ap()
    wk_d = nc.dram_tensor("wk", [P, NDC, K], BF16, kind="ExternalInput").ap()
    bias_d = nc.dram_tensor("bias2", [2, K], BF16, kind="ExternalInput").ap()
    ct_d = nc.dram_tensor("ct", [K, D], F32, kind="ExternalInput").ap()
    out_d = nc.dram_tensor("out", [B_LOC, K * D], F32, kind="ExternalOutput").ap()

    mult = mybir.AluOpType.mult
    add = mybir.AluOpType.add
    AF = mybir.ActivationFunctionType

    with tile.TileContext(nc) as tc:
        from contextlib import ExitStack

        with ExitStack() as ctx:
            singles = ctx.enter_context(tc.tile_pool(name="singles", bufs=1))
            xtin = ctx.enter_context(tc.tile_pool(name="xtin", bufs=4))
            xfin = ctx.enter_context(tc.tile_pool(name="xfin", bufs=4))
            soft = ctx.enter_context(tc.tile_pool(name="soft", bufs=4))
            apool = ctx.enter_context(tc.tile_pool(name="apool", bufs=3))
            fin = ctx.enter_context(tc.tile_pool(name="fin", bufs=2))
            ps = ctx.enter_context(tc.tile_pool(name="ps", bufs=4, space="PSUM"))
            pv = ctx.enter_context(tc.tile_pool(name="pv", bufs=2, space="PSUM"))
            pa = ctx.enter_context(tc.tile_pool(name="pa", bufs=2, space="PSUM"))

            # ---- constants (loaded once) ----
            wk_sb = singles.tile([P, NDC, K], BF16)  # [d_in_chunk, c, k]
            nc.gpsimd.dma_start(out=wk_sb, in_=wk_d)
            bias_sb = singles.tile([2, K], BF16)
            nc.gpsimd.dma_start(out=bias_sb, in_=bias_d)
            ones2 = singles.tile([2, P], BF16)
            nc.vector.memset(ones2[:], 1.0)
            ct_sb = singles.tile([K, D], F32)
            nc.gpsimd.dma_start(out=ct_sb, in_=ct_d)
            # fp8 ones for the DoubleRow asum matmul rhs [128, 2, 1]
            ones8 = singles.tile([P, 2, 1], FP8)
            nc.vector.memset(ones8[:], 1.0)
            c64 = singles.tile([P, K], BF16)
            nc.vector.memset(c64[:], A_SCALE)
            ones_col_f = singles.tile([P, 1], F32)
            nc.vector.memset(ones_col_f[:], 1.0)
            ones_row_f = singles.tile([1, K], F32)
            nc.vector.memset(ones_row_f[:], 1.0)
            eps_sb = singles.tile([K, 1], F32)
            nc.vector.memset(eps_sb[:], EPS)

            def emit_sup_dma(b, si, supers):
                # tail super holds only 192 valid pixels (chunks 36 full, 37
                # half) — skip the zero padding to save DMA bytes
                tail = si == NSUP - 1
                npix = HW - si * SUP if tail else SUP
                xt_sb = xtin.tile([P, NDC, SUP], BF16, tag="xt", name="xt_sb")
                nc.sync.dma_start(
                    out=xt_sb[:, :, :npix],
                    in_=xt_d[b, si].rearrange("p (c q) -> p c q", c=NDC)[
                        :, :, :npix
                    ],
                )
                xf_sb = xfin.tile([P, 4, D], FP8, tag="xf", name="xf_sb")
                nch_sup = (npix + P - 1) // P  # chunk slots with valid pixels
                nc.sync.dma_start(
                    out=xf_sb[:, :nch_sup, :],
                    in_=xf_d[b, si].rearrange("p (c q) -> p c q", c=4)[
                        :, :nch_sup, :
                    ],
                )
                supers["xt"] = xt_sb
                supers["xf"] = xf_sb

            def emit_s(b, pu, state, supers):
                # one chunk-pair: chunks 2u, 2u+1 accumulate into one PSUM bank
                s_ps = ps.tile([P, 2, K], F32, tag="s", name="s_ps")
                for j in range(2):
                    ci = 2 * pu + j
                    p0, psz = CHUNKS[ci]
                    q0 = p0 - (ci // 4) * SUP  # pixel offset within the super
                    xt_sb = supers["xt"]
                    for c in range(NDC):
                        nc.tensor.matmul(
                            s_ps[:psz, j, :],
                            xt_sb[:, c, q0 : q0 + psz],
                            wk_sb[:, c, :],
                            start=(c == 0),
                            stop=False,
                        )
                    # bias add fused into the accumulation group (hi+lo rows)
                    nc.tensor.matmul(
                        s_ps[:psz, j, :], ones2[:, :psz], bias_sb[:],
                        start=False, stop=True,
                    )
                state[pu] = s_ps

            def emit_softmax(pu, state, aready):
                s_ps = state.pop(pu)
                tail = (2 * pu + 1 == NCH - 1) and CHUNKS[-1][1] < P
                e_sb = soft.tile([P, 2, K], BF16, tag="e", name="e_sb")
                # one Exp for both chunks of the pair (no accum: the ACT
                # accumulator read costs ~187ns/inst; r comes from DVE)
                nc.scalar.activation(e_sb[:], s_ps[:], AF.Exp)
                # bf16 r: one rounding of a fp32-accumulated sum (~0.4%), far
                # below the fp8 a-quantization noise; keeps the reduce in the
                # DVE 2x 16-bit mode
                r2 = soft.tile([P, 2], BF16, tag="r", name="r2")
                with nc.allow_low_precision("r rounding ≪ fp8 a-quant noise"):
                    nc.vector.reduce_sum(r2[:], e_sb[:], axis=mybir.AxisListType.X)
                rinv2 = soft.tile([P, 2], F32, tag="rinv", name="rinv2")
                nc.vector.reciprocal(rinv2[:], r2[:])
                # a64 = e * rinv * 64, quantized to fp8 (scale cancels in norm)
                a_t = apool.tile([P, 2, K], FP8, tag="a", name="a_pair")
                for j, eng in ((0, nc.vector), (1, nc.vector)):
                    psz = CHUNKS[2 * pu + j][1]
                    eng.scalar_tensor_tensor(
                        out=a_t[:psz, j, :],
                        in0=e_sb[:psz, j, :],
                        scalar=rinv2[:psz, j : j + 1],
                        in1=c64[:psz],
                        op0=mult,
                        op1=mult,
                    )
                if tail:
                    psz = CHUNKS[-1][1]
                    nc.vector.memset(a_t[psz:, 1, :], 0.0)
                aready.append(a_t)

            def emit_v(pi, vt, asum_ps, aready, xf_list):
                a_t = aready.pop(0)
                xf_sb, cbase = xf_list.pop(0)
                nc.tensor.matmul(
                    vt[:],
                    a_t[:],
                    xf_sb[:, cbase : cbase + 2, :],
                    start=(pi == 0),
                    stop=(pi == NPAIR - 1),
                    perf_mode=mybir.MatmulPerfMode.DoubleRow,
                )
                nc.tensor.matmul(
                    asum_ps[:],
                    a_t[:],
                    ones8[:],
                    start=(pi == 0),
                    stop=(pi == NPAIR - 1),
                    perf_mode=mybir.MatmulPerfMode.DoubleRow,
                )

            def finalize(b, vt_ps, pa_t):
                # pa_t: [K, 4]: col0 = asum (accumulated), col1 = g, col2 = g bcast
                asum_sb = fin.tile([K, 1], F32, tag="asum_sb")
                # a was scaled by 64; C term needs matching scale
                nc.scalar.copy(out=asum_sb[:], in_=pa_t[:, 0:1])
                # vT[k, d] = vt_ps + asum[k] * C[d, k]   (both 64x-scaled)
                vt_sb = fin.tile([K, D], F32, tag="vt")
                nc.vector.scalar_tensor_tensor(
                    out=vt_sb[:],
                    in0=ct_sb[:],
                    scalar=asum_sb[:],
                    in1=vt_ps[:],
                    op0=mult,
                    op1=add,
                )
                # intra-norm: nsq[k] = sum_d vT[k,d]^2 (square+reduce fused)
                sq_sb = fin.tile([K, D], F32, tag="sq")
                nsq = fin.tile([K, 1], F32, tag="nsq")
                nc.vector.tensor_tensor_reduce(
                    out=sq_sb[:],
                    in0=vt_sb[:],
                    in1=vt_sb[:],
                    scale=1.0,
                    scalar=0.0,
                    op0=mult,
                    op1=add,
                    accum_out=nsq[:],
                )
                # rnorm = 1/sqrt(nsq+eps) = exp(-0.5*ln(nsq+eps))
                lnn = fin.tile([K, 1], F32, tag="lnn")
                nc.scalar.activation(lnn[:], nsq[:], AF.Ln, bias=eps_sb[:])
                rnorm = fin.tile([K, 1], F32, tag="rnorm")
                nc.scalar.activation(rnorm[:], lnn[:], AF.Exp, scale=-0.5)
                # srow = nsq * rnorm^2  (post-intra-norm row energy)
                srow = fin.tile([K, 1], F32, tag="srow")
                nc.vector.scalar_tensor_tensor(
                    out=srow[:], in0=rnorm[:], scalar=nsq[:], in1=rnorm[:],
                    op0=mult, op1=mult,
                )
                # g = sum_k srow -> pa_t[0,1]; broadcast to [K,1] -> pa_t[:,2]
                nc.tensor.matmul(
                    pa_t[0:1, 1:2], srow[:], ones_col_f[:K],
                    start=True, stop=True,
                )
                g_sb = fin.tile([1, 1], F32, tag="g_sb")
                nc.scalar.copy(out=g_sb[:], in_=pa_t[0:1, 1:2])
                nc.tensor.matmul(
                    pa_t[:, 2:3], ones_row_f[:], g_sb[:],
                    start=True, stop=True,
                )
                lng = fin.tile([K, 1], F32, tag="lng")
                nc.scalar.activation(
                    lng[:], pa_t[:, 2:3], AF.Ln, bias=eps_sb[:]
                )
                ginv = fin.tile([K, 1], F32, tag="ginv")
                nc.scalar.activation(ginv[:], lng[:], AF.Exp, scale=-0.5)
                scl = fin.tile([K, 1], F32, tag="scl")
                nc.vector.tensor_mul(scl[:], rnorm[:], ginv[:])
                o_sb = fin.tile([K, D], F32, tag="o")
                nc.vector.tensor_scalar_mul(o_sb[:], vt_sb[:], scl[:])
                nc.scalar.dma_start(
                    out=out_d[b].rearrange("(k d) -> k d", d=D), in_=o_sb[:]
                )

            def body():
                for b in range(B_LOC):
                    vt = pv.tile([K, D], F32, tag="vt_ps", name="vt_ps")
                    pa_t = pa.tile([K, 4], F32, tag="pa_t", name="pa_t")
                    state = {}
                    supers = {}
                    aready = []
                    xf_list = []
                    # pair-level pipeline: s(u) | softmax(u-1) | v(u-2)
                    for u in range(NPAIR + 2):
                        if u % 2 == 0 and u < NPAIR:
                            emit_sup_dma(b, u // 2, supers)
                        if u < NPAIR:
                            emit_s(b, u, state, supers)
                            xf_list.append((supers["xf"], 2 * (u % 2)))
                        if 0 <= u - 1 < NPAIR:
                            emit_softmax(u - 1, state, aready)
                        if u >= 2:
                            emit_v(u - 2, vt[:], pa_t[:, 0:1], aready, xf_list)
                    finalize(b, vt, pa_t)

            if reps == 1:
                body()
            else:
                with tc.For_i(0, reps, 1):
                    body()

    nc.compile()
    return nc


_NC_CACHE = {}


def _get_nc(reps: int = 1):
    if reps not in _NC_CACHE:
        _NC_CACHE[reps] = build_netvlad(reps)
    return _NC_CACHE[reps]


def _make_in_maps(x, kernel, bias, C):
    wk = np.ascontiguousarray(kernel.reshape(D, K)).astype(ml_dtypes.bfloat16)
    wk_t = np.ascontiguousarray(
        wk.reshape(NDC, P, K).transpose(1, 0, 2)
    )  # [p_d, c, k]
    bias_f = np.asarray(bias, dtype=np.float32).reshape(K)
    b_hi = bias_f.astype(ml_dtypes.bfloat16)
    b_lo = (bias_f - b_hi.astype(np.float32)).astype(ml_dtypes.bfloat16)
    bias2 = np.ascontiguousarray(np.stack([b_hi, b_lo], axis=0))
    ct = np.ascontiguousarray(C.reshape(D, K).T, dtype=np.float32)

    xf32 = np.asarray(x, dtype=np.float32).reshape(B, HW, D)
    pad = NSUP * SUP - HW  # 320
    xp = np.concatenate(
        [xf32, np.zeros((B, pad, D), np.float32)], axis=1
    )  # [B, 5120, D]
    # xt: D-major bf16: [b, sup, p_d, c_d, q_pix] = x[b, sup*512+q, c_d*128+p_d]
    xt = (
        xp.reshape(B, NSUP, SUP, NDC, P)
        .transpose(0, 1, 4, 3, 2)
        .astype(ml_dtypes.bfloat16)
        .reshape(B, NSUP, P, NDC * SUP)
    )
    # xf: pixel-major fp8: [b, sup, p_pix, c_pix, q_d] = x[b, sup*512+c*128+p, q_d]
    xf = (
        xp.reshape(B, NSUP, 4, P, D)
        .transpose(0, 1, 3, 2, 4)
        .astype(ml_dtypes.float8_e4m3)
        .reshape(B, NSUP, P, 4 * D)
    )

    in_maps = []
    for i in range(N_CORES):
        sl = slice(i * B_LOC, (i + 1) * B_LOC)
        in_maps.append(
            {
                "xt": np.ascontiguousarray(xt[sl]),
                "xf": np.ascontiguousarray(xf[sl]),
                "wk": wk_t,
                "bias2": bias2,
                "ct": ct,
            }
        )
    return in_maps


def kernel(x, kernel, bias, C):
    """Full-input entry point: x [32,60,80,512], kernel [1,1,512,64],
    bias [1,1,64], C [1,1,1,512,64] -> out [32, 32768] (float32)."""
    nc = _get_nc(reps=1)
    in_maps = _make_in_maps(x, kernel, bias, C)
    res = bass_utils.run_bass_kernel_spmd(nc, in_maps, list(range(N_CORES)))
    out = np.concatenate([res.results[i]["out"] for i in range(N_CORES)], axis=0)
    return out


# revision 26
# speedup vs baseline: 1.0942x; 1.0942x over previous
"""NetVLAD layer kernel for Trainium2 (Bass/Tile), data-parallel over batch on 8 cores.

Math (per image):
  s = x @ Wk + bias          # [HW, K]   x:[HW, D], Wk:[D, K]
  a = softmax(s, axis=-1)    # [HW, K]
  vT[k, d] = sum_p a[p,k] x[p,d] + (sum_p a[p,k]) * C[d,k]
  intra L2-normalize over d -> global L2-normalize -> out [K*D]

Sharding: batch 32 -> 4 images per core; Wk/bias/C replicated.

Layouts/precision (v2 design):
  - Host ships TWO layouts of x: xt (D-major, bf16, pre-tiled for the
    s-matmul stationary) and xf (pixel-major, fp8 e4m3, for the DoubleRow
    v-matmul). No on-chip transposes at all.
  - s-path all bf16 (softmax logits are precision-critical; fp8 fails).
  - v-path fp8: a is quantized to fp8 scaled by 64 (dodges e4m3 subnormals;
    the per-row L2 normalization cancels the scale exactly). v-matmul runs
    in DoubleRow mode: one matmul per pixel-chunk PAIR (2x128 contraction).
"""

import sys

sys.path.insert(0, "/opt/trn_rl_repo")

import numpy as np
import ml_dtypes

import concourse.bacc as bacc
import concourse.bass as bass
import concourse.mybir as mybir
import concourse.tile as tile
from concourse import bass_utils

F32 = mybir.dt.float32
BF16 = mybir.dt.bfloat16
FP8 = mybir.dt.float8e4

N_CORES = 8
B = 32
H, W_IMG, D, K = 60, 80, 512, 64
HW = H * W_IMG            # 4800 pixels per image
B_LOC = B // N_CORES      # 4 images per core
P = 128                   # partition / pixel-chunk size
NDC = D // P              # 4 D-chunks
SUP = 4 * P               # 512-pixel super-chunk
NSUP = (HW + SUP - 1) // SUP          # 10 supers (last partial: 192 px)
CHUNKS = [(i * P, min(P, HW - i * P)) for i in range((HW + P - 1) // P)]
NCH = len(CHUNKS)         # 38 chunks: 37 full + one of 64
NPAIR = (NCH + 1) // 2    # 19 chunk-pairs for the DoubleRow v-matmul

A_SCALE = 64.0            # fp8 quantization scale for a (cancels in norm)
EPS = 1e-12

import os
# ablation level for HW bottleneck attribution: dma < s < soft < full
_ABLATE = os.environ.get("NETVLAD_ABLATE", "full")
_ABL_LEVELS = {"dma": 0, "s": 1, "soft": 2, "v": 3, "full": 4}
_ABL = _ABL_LEVELS[_ABLATE]


class _patched_act_tables:
    """Context manager: force the act-table-load pass to use the one set that
    contains Exp, Ln and Copy, so the kernel never swaps ACT tables. Restores
    the original lookup on exit (it is global concourse state)."""

    def __enter__(self):
        from concourse import hw_specs
        import functools

        self._orig_hw = hw_specs.get_activation_tables
        self._orig_bacc = bacc.get_activation_tables

        orig = self._orig_hw

        @functools.cache
        def patched(arch):
            tabs = dict(orig(arch))
            if "natural_log_exp_and_others" in tabs:
                tabs = {
                    name: (s if name == "natural_log_exp_and_others" else set())
                    for name, s in tabs.items()
                }
            return tabs

        hw_specs.get_activation_tables = patched
        bacc.get_activation_tables = patched

    def __exit__(self, *exc):
        from concourse import hw_specs

        hw_specs.get_activation_tables = self._orig_hw
        bacc.get_activation_tables = self._orig_bacc
        return False


def build_netvlad(reps: int = 1):
    with _patched_act_tables():
        return _build_netvlad_inner(reps)


def _build_netvlad_inner(reps: int):
    nc = bacc.Bacc("TRN2", target_bir_lowering=False, debug=False, num_devices=N_CORES)

    # xt: D-major bf16, pre-tiled per super-chunk: [b, sup, p_d, c_d*512 + q_pix]
    #     element = x[b, pixel=sup*512+q_pix, d=c_d*128+p_d]
    xt_d = nc.dram_tensor(
        "xt", [B_LOC, NSUP, P, NDC * SUP // 1], BF16, kind="ExternalInput"
    ).ap()
    # xf: pixel-major fp8, pre-tiled: [b, sup, p_pix, c_pix*512 + q_d]
    #     element = x[b, pixel=sup*512+c_pix*128+p_pix, d=q_d] (zero-padded tail)
    xf_d = nc.dram_tensor(
        "xf", [B_LOC, NSUP, P, 4 * D], FP8, kind="ExternalInput"
    ).ap()
    wk_d = nc.dram_tensor("wk", [P, NDC, K], BF16, kind="ExternalInput").ap()
    bias_d = nc.dram_tensor("bias2", [2, K], BF16, kind="ExternalInput").ap()
    ct_d = nc.dram_tensor("ct", [K, D], F32, kind="ExternalInput").ap()
    out_d = nc.dram_tensor("out", [B_LOC, K * D], F32, kind="ExternalOutput").ap()

    mult = mybir.AluOpType.mult
    add = mybir.AluOpType.add
    AF = mybir.ActivationFunctionType

    with tile.TileContext(nc) as tc:
        from contextlib import ExitStack

        with ExitStack() as ctx:
            singles = ctx.enter_context(tc.tile_pool(name="singles", bufs=1))
            xtin = ctx.enter_context(tc.tile_pool(name="xtin", bufs=4))
            xfin = ctx.enter_context(tc.tile_pool(name="xfin", bufs=4))
            soft = ctx.enter_context(tc.tile_pool(name="soft", bufs=4))
            apool = ctx.enter_context(tc.tile_pool(name="apool", bufs=3))
            fin = ctx.enter_context(tc.tile_pool(name="fin", bufs=2))
            ps = ctx.enter_context(tc.tile_pool(name="ps", bufs=4, space="PSUM"))
            pv = ctx.enter_context(tc.tile_pool(name="pv", bufs=2, space="PSUM"))
            pa = ctx.enter_context(tc.tile_pool(name="pa", bufs=2, space="PSUM"))

            # ---- constants (loaded once) ----
            wk_sb = singles.tile([P, NDC, K], BF16)  # [d_in_chunk, c, k]
            nc.gpsimd.dma_start(out=wk_sb, in_=wk_d)
            bias_sb = singles.tile([2, K], BF16)
            nc.gpsimd.dma_start(out=bias_sb, in_=bias_d)
            ones2 = singles.tile([2, P], BF16)
            nc.vector.memset(ones2[:], 1.0)
            ct_sb = singles.tile([K, D], F32)
            nc.gpsimd.dma_start(out=ct_sb, in_=ct_d)
            # fp8 ones for the asum matmul rhs [128, 1]
            ones8 = singles.tile([P, 1], FP8)
            nc.vector.memset(ones8[:], 1.0)
            c64 = singles.tile([P, K], BF16)
            nc.vector.memset(c64[:], A_SCALE)
            ones_col_f = singles.tile([P, 1], F32)
            nc.vector.memset(ones_col_f[:], 1.0)
            ones_row_f = singles.tile([1, K], F32)
            nc.vector.memset(ones_row_f[:], 1.0)
            eps_sb = singles.tile([K, 1], F32)
            nc.vector.memset(eps_sb[:], EPS)

            def emit_sup_dma(b, si, supers):
                # tail super holds only 192 valid pixels (chunks 36 full, 37
                # half) — skip the zero padding to save DMA bytes
                tail = si == NSUP - 1
                npix = HW - si * SUP if tail else SUP
                xt_sb = xtin.tile([P, NDC, SUP], BF16, tag="xt", name="xt_sb")
                nc.sync.dma_start(
                    out=xt_sb[:, :, :npix],
                    in_=xt_d[b, si].rearrange("p (c q) -> p c q", c=NDC)[
                        :, :, :npix
                    ],
                )
                xf_sb = xfin.tile([P, 4, D], FP8, tag="xf", name="xf_sb")
                nch_sup = (npix + P - 1) // P  # chunk slots with valid pixels
                nc.sync.dma_start(
                    out=xf_sb[:, :nch_sup, :],
                    in_=xf_d[b, si].rearrange("p (c q) -> p c q", c=4)[
                        :, :nch_sup, :
                    ],
                )
                supers["xt"] = xt_sb
                supers["xf"] = xf_sb

            def emit_s(b, pu, state, supers):
                # one chunk-pair: chunks 2u, 2u+1 accumulate into one PSUM bank
                s_ps = ps.tile([P, 2, K], F32, tag="s", name="s_ps")
                for j in range(2):
                    ci = 2 * pu + j
                    p0, psz = CHUNKS[ci]
                    q0 = p0 - (ci // 4) * SUP  # pixel offset within the super
                    xt_sb = supers["xt"]
                    for c in range(NDC):
                        nc.tensor.matmul(
                            s_ps[:psz, j, :],
                            xt_sb[:, c, q0 : q0 + psz],
                            wk_sb[:, c, :],
                            start=(c == 0),
                            stop=False,
                        )
                    # bias add fused into the accumulation group (hi+lo rows)
                    nc.tensor.matmul(
                        s_ps[:psz, j, :], ones2[:, :psz], bias_sb[:],
                        start=False, stop=True,
                    )
                state[pu] = s_ps

            def emit_softmax(pu, state, aready):
                s_ps = state.pop(pu)
                tail = (2 * pu + 1 == NCH - 1) and CHUNKS[-1][1] < P
                e_sb = soft.tile([P, 2, K], BF16, tag="e", name="e_sb")
                # one Exp for both chunks of the pair (no accum: the ACT
                # accumulator read costs ~187ns/inst; r comes from DVE)
                nc.scalar.activation(e_sb[:], s_ps[:], AF.Exp)
                # bf16 r: one rounding of a fp32-accumulated sum (~0.4%), far
                # below the fp8 a-quantization noise; keeps the reduce in the
                # DVE 2x 16-bit mode
                r2 = soft.tile([P, 2], BF16, tag="r", name="r2")
                with nc.allow_low_precision("r rounding ≪ fp8 a-quant noise"):
                    nc.vector.reduce_sum(r2[:], e_sb[:], axis=mybir.AxisListType.X)
                rinv2 = soft.tile([P, 2], F32, tag="rinv", name="rinv2")
                nc.vector.reciprocal(rinv2[:], r2[:])
                # a64 = e * rinv * 64, quantized to fp8 (scale cancels in norm)
                # 2*K inner stride keeps each j-slot 128B-aligned for LDWEIGHTS
                a_t = apool.tile([P, 2, 2 * K], FP8, tag="a", name="a_pair")
                for j, eng in ((0, nc.vector), (1, nc.vector)):
                    psz = CHUNKS[2 * pu + j][1]
                    eng.scalar_tensor_tensor(
                        out=a_t[:psz, j, :K],
                        in0=e_sb[:psz, j, :],
                        scalar=rinv2[:psz, j : j + 1],
                        in1=c64[:psz],
                        op0=mult,
                        op1=mult,
                    )
                if tail:
                    psz = CHUNKS[-1][1]
                    nc.vector.memset(a_t[psz:, 1, :K], 0.0)
                aready.append(a_t)

            def emit_v(pi, vt, asum_ps, aready, xf_list):
                # plain fp8 matmuls: DoubleRow measured ~5x slower than modeled
                # on HW (interleaved no-FWL LDWEIGHTS dominates)
                a_t = aready.pop(0)
                xf_sb, cbase = xf_list.pop(0)
                for j in range(2):
                    ci = 2 * pi + j
                    nc.tensor.matmul(
                        vt[:],
                        a_t[:, j, :K],
                        xf_sb[:, cbase + j, :],
                        start=(ci == 0),
                        stop=(ci == NCH - 1),
                    )
                    nc.tensor.matmul(
                        asum_ps[:],
                        a_t[:, j, :K],
                        ones8[:],
                        start=(ci == 0),
                        stop=(ci == NCH - 1),
                    )

            def finalize(b, vt_ps, pa_t):
                # pa_t: [K, 4]: col0 = asum (accumulated), col1 = g, col2 = g bcast
                asum_sb = fin.tile([K, 1], F32, tag="asum_sb")
                # a was scaled by 64; C term needs matching scale
                nc.scalar.copy(out=asum_sb[:], in_=pa_t[:, 0:1])
                # vT[k, d] = vt_ps + asum[k] * C[d, k]   (both 64x-scaled)
                vt_sb = fin.tile([K, D], F32, tag="vt")
                nc.vector.scalar_tensor_tensor(
                    out=vt_sb[:],
                    in0=ct_sb[:],
                    scalar=asum_sb[:],
                    in1=vt_ps[:],
                    op0=mult,
                    op1=add,
                )
                # intra-norm: nsq[k] = sum_d vT[k,d]^2
                sq_sb = fin.tile([K, D], F32, tag="sq")
                nsq = fin.tile([K, 1], F32, tag="nsq")
                nc.vector.tensor_mul(sq_sb[:], vt_sb[:], vt_sb[:])
                nc.vector.reduce_sum(nsq[:], sq_sb[:], axis=mybir.AxisListType.X)
                # rnorm = 1/sqrt(nsq+eps) = exp(-0.5*ln(nsq+eps))
                lnn = fin.tile([K, 1], F32, tag="lnn")
                nc.scalar.activation(lnn[:], nsq[:], AF.Ln, bias=eps_sb[:])
                rnorm = fin.tile([K, 1], F32, tag="rnorm")
                nc.scalar.activation(rnorm[:], lnn[:], AF.Exp, scale=-0.5)
                # srow = nsq * rnorm^2  (post-intra-norm row energy)
                srow = fin.tile([K, 1], F32, tag="srow")
                nc.vector.scalar_tensor_tensor(
                    out=srow[:], in0=rnorm[:], scalar=nsq[:], in1=rnorm[:],
                    op0=mult, op1=mult,
                )
                # g = sum_k srow -> pa_t[0,1]; broadcast to [K,1] -> pa_t[:,2]
                nc.tensor.matmul(
                    pa_t[0:1, 1:2], srow[:], ones_col_f[:K],
                    start=True, stop=True,
                )
                g_sb = fin.tile([1, 1], F32, tag="g_sb")
                nc.scalar.copy(out=g_sb[:], in_=pa_t[0:1, 1:2])
                nc.tensor.matmul(
                    pa_t[:, 2:3], ones_row_f[:], g_sb[:],
                    start=True, stop=True,
                )
                lng = fin.tile([K, 1], F32, tag="lng")
                nc.scalar.activation(
                    lng[:], pa_t[:, 2:3], AF.Ln, bias=eps_sb[:]
                )
                ginv = fin.tile([K, 1], F32, tag="ginv")
                nc.scalar.activation(ginv[:], lng[:], AF.Exp, scale=-0.5)
                scl = fin.tile([K, 1], F32, tag="scl")
                nc.vector.tensor_mul(scl[:], rnorm[:], ginv[:])
                o_sb = fin.tile([K, D], F32, tag="o")
                nc.vector.tensor_scalar_mul(o_sb[:], vt_sb[:], scl[:])
                nc.scalar.dma_start(
                    out=out_d[b].rearrange("(k d) -> k d", d=D), in_=o_sb[:]
                )

            def body():
                for b in range(B_LOC):
                    if _ABL >= 3:
                        vt = pv.tile([K, D], F32, tag="vt_ps", name="vt_ps")
                        pa_t = pa.tile([K, 4], F32, tag="pa_t", name="pa_t")
                    else:
                        vt = pa_t = None
                    state = {}
                    supers = {}
                    aready = []
                    xf_list = []
                    # pair-level pipeline: s(u) | softmax(u-1) | v(u-2)
                    for u in range(NPAIR + 2):
                        if u % 2 == 0 and u < NPAIR:
                            emit_sup_dma(b, u // 2, supers)
                        if u < NPAIR and _ABL >= 1:
                            emit_s(b, u, state, supers)
                            xf_list.append((supers["xf"], 2 * (u % 2)))
                        if 0 <= u - 1 < NPAIR and _ABL >= 2:
                            emit_softmax(u - 1, state, aready)
                        if u >= 2 and _ABL >= 3:
                            emit_v(u - 2, vt[:], pa_t[:, 0:1], aready, xf_list)
                    if _ABL >= 4:
                        finalize(b, vt, pa_t)
                    else:
                        nc.scalar.dma_start(
                            out=out_d[b].rearrange("(k d) -> k d", d=D),
                            in_=ct_sb[:],
                        )

            if reps == 1:
                body()
            else:
                with tc.For_i(0, reps, 1):
                    body()

    nc.compile()
    return nc


_NC_CACHE = {}


def _get_nc(reps: int = 1):
    if reps not in _NC_CACHE:
        _NC_CACHE[reps] = build_netvlad(reps)
    return _NC_CACHE[reps]


def _make_in_maps(x, kernel, bias, C):
    wk = np.ascontiguousarray(kernel.reshape(D, K)).astype(ml_dtypes.bfloat16)
    wk_t = np.ascontiguousarray(
        wk.reshape(NDC, P, K).transpose(1, 0, 2)
    )  # [p_d, c, k]
    bias_f = np.asarray(bias, dtype=np.float32).reshape(K)
    b_hi = bias_f.astype(ml_dtypes.bfloat16)
    b_lo = (bias_f - b_hi.astype(np.float32)).astype(ml_dtypes.bfloat16)
    bias2 = np.ascontiguousarray(np.stack([b_hi, b_lo], axis=0))
    ct = np.ascontiguousarray(C.reshape(D, K).T, dtype=np.float32)

    xf32 = np.asarray(x, dtype=np.float32).reshape(B, HW, D)
    pad = NSUP * SUP - HW  # 320
    xp = np.concatenate(
        [xf32, np.zeros((B, pad, D), np.float32)], axis=1
    )  # [B, 5120, D]
    # xt: D-major bf16: [b, sup, p_d, c_d, q_pix] = x[b, sup*512+q, c_d*128+p_d]
    xt = (
        xp.reshape(B, NSUP, SUP, NDC, P)
        .transpose(0, 1, 4, 3, 2)
        .astype(ml_dtypes.bfloat16)
        .reshape(B, NSUP, P, NDC * SUP)
    )
    # xf: pixel-major fp8: [b, sup, p_pix, c_pix, q_d] = x[b, sup*512+c*128+p, q_d]
    xf = (
        xp.reshape(B, NSUP, 4, P, D)
        .transpose(0, 1, 3, 2, 4)
        .astype(ml_dtypes.float8_e4m3)
        .reshape(B, NSUP, P, 4 * D)
    )

    in_maps = []
    for i in range(N_CORES):
        sl = slice(i * B_LOC, (i + 1) * B_LOC)
        in_maps.append(
            {
                "xt": np.ascontiguousarray(xt[sl]),
                "xf": np.ascontiguousarray(xf[sl]),
                "wk": wk_t,
                "bias2": bias2,
                "ct": ct,
            }
        )
    return in_maps


def kernel(x, kernel, bias, C):
    """Full-input entry point: x [32,60,80,512], kernel [1,1,512,64],
    bias [1,1,64], C [1,1,1,512,64] -> out [32, 32768] (float32)."""
    nc = _get_nc(reps=1)
    in_maps = _make_in_maps(x, kernel, bias, C)
    res = bass_utils.run_bass_kernel_spmd(nc, in_maps, list(range(N_CORES)))
    out = np.concatenate([res.results[i]["out"] for i in range(N_CORES)], axis=0)
    return out


# revision 29
# speedup vs baseline: 1.2777x; 1.1676x over previous
"""NetVLAD layer kernel for Trainium2 (Bass/Tile), data-parallel over batch on 8 cores.

Math (per image):
  s = x @ Wk + bias          # [HW, K]   x:[HW, D], Wk:[D, K]
  a = softmax(s, axis=-1)    # [HW, K]
  vT[k, d] = sum_p a[p,k] x[p,d] + (sum_p a[p,k]) * C[d,k]
  intra L2-normalize over d -> global L2-normalize -> out [K*D]

Sharding: batch 32 -> 4 images per core; Wk/bias/C replicated.

Layouts/precision (v2 design):
  - Host ships TWO layouts of x: xt (D-major, bf16, pre-tiled for the
    s-matmul stationary) and xf (pixel-major, fp8 e4m3, for the DoubleRow
    v-matmul). No on-chip transposes at all.
  - s-path all bf16 (softmax logits are precision-critical; fp8 fails).
  - v-path fp8: a is quantized to fp8 scaled by 64 (dodges e4m3 subnormals;
    the per-row L2 normalization cancels the scale exactly). v-matmul runs
    in DoubleRow mode: one matmul per pixel-chunk PAIR (2x128 contraction).
"""

import sys

sys.path.insert(0, "/opt/trn_rl_repo")

import numpy as np
import ml_dtypes

import concourse.bacc as bacc
import concourse.bass as bass
import concourse.mybir as mybir
import concourse.tile as tile
from concourse import bass_utils

F32 = mybir.dt.float32
BF16 = mybir.dt.bfloat16
FP8 = mybir.dt.float8e4

N_CORES = 8
B = 32
H, W_IMG, D, K = 60, 80, 512, 64
HW = H * W_IMG            # 4800 pixels per image
B_LOC = B // N_CORES      # 4 images per core
P = 128                   # partition / pixel-chunk size
NDC = D // P              # 4 D-chunks
SUP = 4 * P               # 512-pixel super-chunk
NSUP = (HW + SUP - 1) // SUP          # 10 supers (last partial: 192 px)
CHUNKS = [(i * P, min(P, HW - i * P)) for i in range((HW + P - 1) // P)]
NCH = len(CHUNKS)         # 38 chunks: 37 full + one of 64
NPAIR = (NCH + 1) // 2    # 19 chunk-pairs for the DoubleRow v-matmul

A_SCALE = 64.0            # fp8 quantization scale for a (cancels in norm)
EPS = 1e-12

import os
# ablation level for HW bottleneck attribution: dma < s < soft < full
_ABLATE = os.environ.get("NETVLAD_ABLATE", "full")
_ABL_LEVELS = {"dma": 0, "s": 1, "soft": 2, "v": 3, "full": 4}
_ABL = _ABL_LEVELS[_ABLATE]


class _patched_act_tables:
    """Context manager: force the act-table-load pass to use the one set that
    contains Exp, Ln and Copy, so the kernel never swaps ACT tables. Restores
    the original lookup on exit (it is global concourse state)."""

    def __enter__(self):
        from concourse import hw_specs
        import functools

        self._orig_hw = hw_specs.get_activation_tables
        self._orig_bacc = bacc.get_activation_tables

        orig = self._orig_hw

        @functools.cache
        def patched(arch):
            tabs = dict(orig(arch))
            if "natural_log_exp_and_others" in tabs:
                tabs = {
                    name: (s if name == "natural_log_exp_and_others" else set())
                    for name, s in tabs.items()
                }
            return tabs

        hw_specs.get_activation_tables = patched
        bacc.get_activation_tables = patched

    def __exit__(self, *exc):
        from concourse import hw_specs

        hw_specs.get_activation_tables = self._orig_hw
        bacc.get_activation_tables = self._orig_bacc
        return False


def build_netvlad(reps: int = 1):
    with _patched_act_tables():
        return _build_netvlad_inner(reps)


def _build_netvlad_inner(reps: int):
    nc = bacc.Bacc("TRN2", target_bir_lowering=False, debug=False, num_devices=N_CORES)

    # xt: D-major bf16, pre-tiled per super-chunk: [b, sup, p_d, c_d*512 + q_pix]
    #     element = x[b, pixel=sup*512+q_pix, d=c_d*128+p_d]
    xt_d = nc.dram_tensor(
        "xt", [B_LOC, NSUP, P, NDC * SUP // 1], BF16, kind="ExternalInput"
    ).ap()
    # xf: pixel-major fp8, pre-tiled: [b, sup, p_pix, c_pix*512 + q_d]
    #     element = x[b, pixel=sup*512+c_pix*128+p_pix, d=q_d] (zero-padded tail)
    xf_d = nc.dram_tensor(
        "xf", [B_LOC, NSUP, P, 4 * D], FP8, kind="ExternalInput"
    ).ap()
    wk_d = nc.dram_tensor("wk", [P, NDC, K], BF16, kind="ExternalInput").ap()
    bias_d = nc.dram_tensor("bias2", [2, K], BF16, kind="ExternalInput").ap()
    ct_d = nc.dram_tensor("ct", [K, D], F32, kind="ExternalInput").ap()
    out_d = nc.dram_tensor("out", [B_LOC, K * D], F32, kind="ExternalOutput").ap()

    mult = mybir.AluOpType.mult
    add = mybir.AluOpType.add
    AF = mybir.ActivationFunctionType

    with tile.TileContext(nc) as tc:
        from contextlib import ExitStack

        with ExitStack() as ctx:
            singles = ctx.enter_context(tc.tile_pool(name="singles", bufs=1))
            xtin = ctx.enter_context(tc.tile_pool(name="xtin", bufs=4))
            xfin = ctx.enter_context(tc.tile_pool(name="xfin", bufs=5))
            soft = ctx.enter_context(tc.tile_pool(name="soft", bufs=5))
            apool = ctx.enter_context(tc.tile_pool(name="apool", bufs=5))
            fin = ctx.enter_context(tc.tile_pool(name="fin", bufs=2))
            ps = ctx.enter_context(tc.tile_pool(name="ps", bufs=4, space="PSUM"))
            pv = ctx.enter_context(tc.tile_pool(name="pv", bufs=2, space="PSUM"))
            pa = ctx.enter_context(tc.tile_pool(name="pa", bufs=2, space="PSUM"))

            # ---- constants (loaded once) ----
            wk_sb = singles.tile([P, NDC, K], BF16)  # [d_in_chunk, c, k]
            nc.gpsimd.dma_start(out=wk_sb, in_=wk_d)
            bias_sb = singles.tile([2, K], BF16)
            nc.gpsimd.dma_start(out=bias_sb, in_=bias_d)
            ones2 = singles.tile([2, P], BF16)
            nc.vector.memset(ones2[:], 1.0)
            ct_sb = singles.tile([K, D], F32)
            nc.gpsimd.dma_start(out=ct_sb, in_=ct_d)
            # fp8 ones for the asum matmul rhs [128, 1]
            ones8 = singles.tile([P, 1], FP8)
            nc.vector.memset(ones8[:], 1.0)
            c64 = singles.tile([P, K], BF16)
            nc.vector.memset(c64[:], A_SCALE)
            ones_col_f = singles.tile([P, 1], F32)
            nc.vector.memset(ones_col_f[:], 1.0)
            ones_row_f = singles.tile([1, K], F32)
            nc.vector.memset(ones_row_f[:], 1.0)
            eps_sb = singles.tile([K, 1], F32)
            nc.vector.memset(eps_sb[:], EPS)

            def emit_sup_dma(b, si, supers):
                # tail super holds only 192 valid pixels (chunks 36 full, 37
                # half) — skip the zero padding to save DMA bytes
                tail = si == NSUP - 1
                npix = HW - si * SUP if tail else SUP
                xt_sb = xtin.tile([P, NDC, SUP], BF16, tag="xt", name="xt_sb")
                nc.sync.dma_start(
                    out=xt_sb[:, :, :npix],
                    in_=xt_d[b, si].rearrange("p (c q) -> p c q", c=NDC)[
                        :, :, :npix
                    ],
                )
                xf_sb = xfin.tile([P, 4, D], FP8, tag="xf", name="xf_sb")
                nch_sup = (npix + P - 1) // P  # chunk slots with valid pixels
                nc.sync.dma_start(
                    out=xf_sb[:, :nch_sup, :],
                    in_=xf_d[b, si].rearrange("p (c q) -> p c q", c=4)[
                        :, :nch_sup, :
                    ],
                )
                supers["xt"] = xt_sb
                supers["xf"] = xf_sb

            def emit_s(b, pu, state, supers):
                # one chunk-pair: chunks 2u, 2u+1 accumulate into one PSUM bank
                s_ps = ps.tile([P, 2, K], F32, tag="s", name="s_ps")
                for j in range(2):
                    ci = 2 * pu + j
                    p0, psz = CHUNKS[ci]
                    q0 = p0 - (ci // 4) * SUP  # pixel offset within the super
                    xt_sb = supers["xt"]
                    for c in range(NDC):
                        nc.tensor.matmul(
                            s_ps[:psz, j, :],
                            xt_sb[:, c, q0 : q0 + psz],
                            wk_sb[:, c, :],
                            start=(c == 0),
                            stop=False,
                        )
                    # bias add fused into the accumulation group (hi+lo rows)
                    nc.tensor.matmul(
                        s_ps[:psz, j, :], ones2[:, :psz], bias_sb[:],
                        start=False, stop=True,
                    )
                state[pu] = s_ps

            def emit_softmax(pu, state, aready):
                # per-chunk softmax with fused Exp+accum: the cross-engine
                # dependency chain s->Exp->recip->stt->v gates the PE; one
                # fewer hop beats saving the 187ns ACT accumulator read
                s_ps = state.pop(pu)
                pair_as = []
                for j in range(2):
                    psz = CHUNKS[2 * pu + j][1]
                    e_sb = soft.tile([P, K], BF16, tag=f"e{j}", name="e_sb")
                    r_sb = soft.tile([P, 1], F32, tag=f"r{j}", name="r_sb")
                    nc.scalar.activation(
                        e_sb[:psz], s_ps[:psz, j, :], AF.Exp, accum_out=r_sb[:psz]
                    )
                    rinv = soft.tile([P, 1], F32, tag=f"ri{j}", name="rinv")
                    nc.vector.reciprocal(rinv[:psz], r_sb[:psz])
                    # a64 = e * rinv * 64 -> fp8 (scale cancels in the norm)
                    a_c = apool.tile([P, K], FP8, tag=f"a{j}", name="a_c")
                    nc.vector.scalar_tensor_tensor(
                        out=a_c[:psz],
                        in0=e_sb[:psz],
                        scalar=rinv[:psz],
                        in1=c64[:psz],
                        op0=mult,
                        op1=mult,
                    )
                    if psz < P:
                        nc.vector.memset(a_c[psz:], 0.0)
                    pair_as.append(a_c)
                aready.append(pair_as)

            def emit_v(pi, vt, asum_ps, aready, xf_list):
                # plain fp8 matmuls: DoubleRow measured ~5x slower than modeled
                # on HW (interleaved no-FWL LDWEIGHTS dominates)
                pair_as = aready.pop(0)
                xf_sb, cbase = xf_list.pop(0)
                for j in range(2):
                    ci = 2 * pi + j
                    nc.tensor.matmul(
                        vt[:],
                        pair_as[j][:],
                        xf_sb[:, cbase + j, :],
                        start=(ci == 0),
                        stop=(ci == NCH - 1),
                    )
                    nc.tensor.matmul(
                        asum_ps[:],
                        pair_as[j][:],
                        ones8[:],
                        start=(ci == 0),
                        stop=(ci == NCH - 1),
                    )

            def finalize(b, vt_ps, pa_t):
                # pa_t: [K, 4]: col0 = asum (accumulated), col1 = g, col2 = g bcast
                asum_sb = fin.tile([K, 1], F32, tag="asum_sb")
                # a was scaled by 64; C term needs matching scale
                nc.scalar.copy(out=asum_sb[:], in_=pa_t[:, 0:1])
                # vT[k, d] = vt_ps + asum[k] * C[d, k]   (both 64x-scaled)
                vt_sb = fin.tile([K, D], F32, tag="vt")
                nc.vector.scalar_tensor_tensor(
                    out=vt_sb[:],
                    in0=ct_sb[:],
                    scalar=asum_sb[:],
                    in1=vt_ps[:],
                    op0=mult,
                    op1=add,
                )
                # intra-norm: nsq[k] = sum_d vT[k,d]^2
                sq_sb = fin.tile([K, D], F32, tag="sq")
                nsq = fin.tile([K, 1], F32, tag="nsq")
                nc.vector.tensor_mul(sq_sb[:], vt_sb[:], vt_sb[:])
                nc.vector.reduce_sum(nsq[:], sq_sb[:], axis=mybir.AxisListType.X)
                # rnorm = 1/sqrt(nsq+eps) = exp(-0.5*ln(nsq+eps))
                lnn = fin.tile([K, 1], F32, tag="lnn")
                nc.scalar.activation(lnn[:], nsq[:], AF.Ln, bias=eps_sb[:])
                rnorm = fin.tile([K, 1], F32, tag="rnorm")
                nc.scalar.activation(rnorm[:], lnn[:], AF.Exp, scale=-0.5)
                # srow = nsq * rnorm^2  (post-intra-norm row energy)
                srow = fin.tile([K, 1], F32, tag="srow")
                nc.vector.scalar_tensor_tensor(
                    out=srow[:], in0=rnorm[:], scalar=nsq[:], in1=rnorm[:],
                    op0=mult, op1=mult,
                )
                # g = sum_k srow -> pa_t[0,1]; broadcast to [K,1] -> pa_t[:,2]
                nc.tensor.matmul(
                    pa_t[0:1, 1:2], srow[:], ones_col_f[:K],
                    start=True, stop=True,
                )
                g_sb = fin.tile([1, 1], F32, tag="g_sb")
                nc.scalar.copy(out=g_sb[:], in_=pa_t[0:1, 1:2])
                nc.tensor.matmul(
                    pa_t[:, 2:3], ones_row_f[:], g_sb[:],
                    start=True, stop=True,
                )
                lng = fin.tile([K, 1], F32, tag="lng")
                nc.scalar.activation(
                    lng[:], pa_t[:, 2:3], AF.Ln, bias=eps_sb[:]
                )
                ginv = fin.tile([K, 1], F32, tag="ginv")
                nc.scalar.activation(ginv[:], lng[:], AF.Exp, scale=-0.5)
                scl = fin.tile([K, 1], F32, tag="scl")
                nc.vector.tensor_mul(scl[:], rnorm[:], ginv[:])
                o_sb = fin.tile([K, D], F32, tag="o")
                nc.vector.tensor_scalar_mul(o_sb[:], vt_sb[:], scl[:])
                nc.scalar.dma_start(
                    out=out_d[b].rearrange("(k d) -> k d", d=D), in_=o_sb[:]
                )

            def body():
                for b in range(B_LOC):
                    if _ABL >= 3:
                        vt = pv.tile([K, D], F32, tag="vt_ps", name="vt_ps")
                        pa_t = pa.tile([K, 4], F32, tag="pa_t", name="pa_t")
                    else:
                        vt = pa_t = None
                    state = {}
                    supers = {}
                    aready = []
                    xf_list = []
                    # pair-level pipeline: s(u) | softmax(u-1) | v(u-3)
                    VLAG = 3
                    for u in range(NPAIR + VLAG):
                        if u % 2 == 0 and u < NPAIR:
                            emit_sup_dma(b, u // 2, supers)
                        if u < NPAIR and _ABL >= 1:
                            emit_s(b, u, state, supers)
                            xf_list.append((supers["xf"], 2 * (u % 2)))
                        if 0 <= u - 1 < NPAIR and _ABL >= 2:
                            emit_softmax(u - 1, state, aready)
                        if u >= VLAG and _ABL >= 3:
                            emit_v(u - VLAG, vt[:], pa_t[:, 0:1], aready, xf_list)
                    if _ABL >= 4:
                        finalize(b, vt, pa_t)
                    else:
                        nc.scalar.dma_start(
                            out=out_d[b].rearrange("(k d) -> k d", d=D),
                            in_=ct_sb[:],
                        )

            if reps == 1:
                body()
            else:
                with tc.For_i(0, reps, 1):
                    body()

    nc.compile()
    return nc


_NC_CACHE = {}


def _get_nc(reps: int = 1):
    if reps not in _NC_CACHE:
        _NC_CACHE[reps] = build_netvlad(reps)
    return _NC_CACHE[reps]


def _make_in_maps(x, kernel, bias, C):
    wk = np.ascontiguousarray(kernel.reshape(D, K)).astype(ml_dtypes.bfloat16)
    wk_t = np.ascontiguousarray(
        wk.reshape(NDC, P, K).transpose(1, 0, 2)
    )  # [p_d, c, k]
    bias_f = np.asarray(bias, dtype=np.float32).reshape(K)
    b_hi = bias_f.astype(ml_dtypes.bfloat16)
    b_lo = (bias_f - b_hi.astype(np.float32)).astype(ml_dtypes.bfloat16)
    bias2 = np.ascontiguousarray(np.stack([b_hi, b_lo], axis=0))
    ct = np.ascontiguousarray(C.reshape(D, K).T, dtype=np.float32)

    xf32 = np.asarray(x, dtype=np.float32).reshape(B, HW, D)
    pad = NSUP * SUP - HW  # 320
    xp = np.concatenate(
        [xf32, np.zeros((B, pad, D), np.float32)], axis=1
    )  # [B, 5120, D]
    # xt: D-major bf16: [b, sup, p_d, c_d, q_pix] = x[b, sup*512+q, c_d*128+p_d]
    xt = (
        xp.reshape(B, NSUP, SUP, NDC, P)
        .transpose(0, 1, 4, 3, 2)
        .astype(ml_dtypes.bfloat16)
        .reshape(B, NSUP, P, NDC * SUP)
    )
    # xf: pixel-major fp8: [b, sup, p_pix, c_pix, q_d] = x[b, sup*512+c*128+p, q_d]
    xf = (
        xp.reshape(B, NSUP, 4, P, D)
        .transpose(0, 1, 3, 2, 4)
        .astype(ml_dtypes.float8_e4m3)
        .reshape(B, NSUP, P, 4 * D)
    )

    in_maps = []
    for i in range(N_CORES):
        sl = slice(i * B_LOC, (i + 1) * B_LOC)
        in_maps.append(
            {
                "xt": np.ascontiguousarray(xt[sl]),
                "xf": np.ascontiguousarray(xf[sl]),
                "wk": wk_t,
                "bias2": bias2,
                "ct": ct,
            }
        )
    return in_maps


def kernel(x, kernel, bias, C):
    """Full-input entry point: x [32,60,80,512], kernel [1,1,512,64],
    bias [1,1,64], C [1,1,1,512,64] -> out [32, 32768] (float32)."""
    nc = _get_nc(reps=1)
    in_maps = _make_in_maps(x, kernel, bias, C)
    res = bass_utils.run_bass_kernel_spmd(nc, in_maps, list(range(N_CORES)))
    out = np.concatenate([res.results[i]["out"] for i in range(N_CORES)], axis=0)
    return out
